# revision 3
# baseline (speedup 1.0000x reference)
"""Trainium2 Bass kernel for nn_Attention_85581518340337.

Restormer-style channel attention:
  x (1,64,16,64,64) -> 1x1x1 conv (64->768) -> grouped 3x3x3 conv (192 groups of 4)
  -> split q,k,v (4 heads x 64 ch) -> L2 normalize over n=t*h*w -> attn = softmax(q@kT * temp)
  -> out = attn@v -> 1x1x1 proj (256->64)

Sharding: spatial over H (64 rows -> 8 cores x 8 rows, halo 1 row each side).
Per core: folded (qkv1*dwconv) dense conv for q,k computed in FP8 with DoubleRow
matmuls (4 taps per MM: 2 via the shifted partition bands of xa8/xb8, x2 via the
DoubleRow ko dimension expressed as a shifted 4-dim access pattern); per-output-
channel weight scales cancel exactly in the L2 normalization. q,k DMA-transposed
(bf16) and reduced to per-head Gram matrices on PE; one 133KB AllReduce of Gram
partials; softmax + projection folded into a per-head 64x64 matrix B_h;
out = concat_h(B_h) @ v computed as a bf16 conv (G = B o Mfold_v) col-tiled
2 t-planes at a time (M=64 per column group).
"""

import numpy as np
import ml_dtypes

import concourse.bass as bass
import concourse.mybir as mybir
import concourse.tile as tile
from concourse import bacc
from concourse.bass_utils import run_bass_kernel_spmd

F32 = mybir.dt.float32
BF16 = mybir.dt.bfloat16
FP8 = mybir.dt.float8e4
DR = mybir.MatmulPerfMode.DoubleRow

N_CORES = 8
DIM = 64
HEADS = 4
T, H, W = 16, 64, 64
HL = H // N_CORES          # 8 output h-rows per core
HLH = HL + 2               # 10 h-rows incl halo
C3H = DIM * 3 * HEADS      # 768
N_LOC = T * HL * W         # 8192 output positions per core
NT = 512                   # matmul free tile = one t-plane (8*64)

# padded staging layout (t, h, w) = (18, 10, 68)
PT, PH, PW = T + 2, HLH, W + 4
PLANE = PH * PW            # 680
PFREE = PT * PLANE         # 12240

# FP8 DoubleRow slot table for the folded q,k conv: 8 MMs per t-plane.
# Each MM contracts K=256 = 2 bands (partition halves of xa8/xb8) x 2 ko
# (DoubleRow k-tiles, expressed as a window shifted by `delta` elements).
# (tensor: 0=xa8 (bands w+0/w+1), 1=xb8 (bands h+0/h+1), dt, h0, w0, delta)
DR_SLOTS = [
    (0, 0, 0, 1, PW),       # s0: taps (0,{0,1},{0,1})
    (0, 1, 0, 1, PW),       # s1: taps (1,{0,1},{0,1})
    (0, 2, 0, 1, PW),       # s2: taps (2,{0,1},{0,1})
    (0, 0, 2, 1, PLANE),    # s3: taps ({0,1},2,{0,1})
    (0, 2, 2, 1, 2),        # s4: taps (2,2,{0,1,2}) + phantom
    (1, 0, 0, 3, PLANE),    # s5: taps ({0,1},{0,1},2)
    (1, 2, 0, 3, 2 * PW),   # s6: taps (2,{0,1},2) + 2 phantom
    (1, 0, 2, 3, PLANE),    # s7: taps ({0,1},2,2) + 2 phantom
]
# taps[s][ko][band] -> (dt, dh, dw) or None (zero weight)
DR_TAPS = [
    [[(0, 0, 0), (0, 0, 1)], [(0, 1, 0), (0, 1, 1)]],
    [[(1, 0, 0), (1, 0, 1)], [(1, 1, 0), (1, 1, 1)]],
    [[(2, 0, 0), (2, 0, 1)], [(2, 1, 0), (2, 1, 1)]],
    [[(0, 2, 0), (0, 2, 1)], [(1, 2, 0), (1, 2, 1)]],
    [[(2, 2, 0), (2, 2, 1)], [(2, 2, 2), None]],
    [[(0, 0, 2), (0, 1, 2)], [(1, 0, 2), (1, 1, 2)]],
    [[(2, 0, 2), (2, 1, 2)], [None, None]],
    [[(0, 2, 2), None], [(1, 2, 2), None]],
]

_CACHE = {}


def _build(sim=False, stop_after=None, reps=0, local_ar=False):
    nc = bacc.Bacc("TRN2", target_bir_lowering=False, debug=False,
                   num_devices=1 if sim else N_CORES)

    x_d = nc.dram_tensor("x", [DIM, PFREE], BF16, kind="ExternalInput").ap()
    x8_d = nc.dram_tensor("x8", [DIM, PFREE], FP8, kind="ExternalInput").ap()
    dwt8_d = nc.dram_tensor("dwt8", [4, 128, 2048], FP8, kind="ExternalInput").ap()
    dwtv_d = nc.dram_tensor("dwtv", [14, 2, 128, 128], BF16, kind="ExternalInput").ap()
    projt_d = nc.dram_tensor("projt", [128, 2, DIM], F32, kind="ExternalInput").ap()
    temp_d = nc.dram_tensor("temp", [HEADS], F32, kind="ExternalInput").ap()
    eye_d = nc.dram_tensor("eye", [128, 128], F32, kind="ExternalInput").ap()
    out_d = nc.dram_tensor("out", [DIM, T, HL, W], F32, kind="ExternalOutput").ap()

    with tile.TileContext(nc) as tc:
        if reps:
            with tc.For_i(0, reps):
                _emit(nc, tc, x_d, x8_d, dwt8_d, dwtv_d, projt_d, temp_d, eye_d, out_d,
                      sim=sim or local_ar, stop_after=stop_after)
        else:
            _emit(nc, tc, x_d, x8_d, dwt8_d, dwtv_d, projt_d, temp_d, eye_d, out_d,
                  sim=sim or local_ar, stop_after=stop_after)
    nc.compile()
    return nc


def _emit(nc, tc, x_d, x8_d, dwt8_d, dwtv_d, projt_d, temp_d, eye_d, out_d,
          sim=False, stop_after=None):
    import contextlib
    ctx = contextlib.ExitStack()
    with ctx:
        singles = ctx.enter_context(tc.tile_pool(name="singles", bufs=1))
        dwt_p = ctx.enter_context(tc.tile_pool(name="dwtp", bufs=2))
        dense_p = ctx.enter_context(tc.tile_pool(name="dense", bufs=1))
        ct_p = ctx.enter_context(tc.tile_pool(name="ctp", bufs=4))
        small_p = ctx.enter_context(tc.tile_pool(name="small", bufs=2))
        out_p = ctx.enter_context(tc.tile_pool(name="outp", bufs=3))
        ps_conv = ctx.enter_context(tc.tile_pool(name="ps_conv", bufs=4, space="PSUM"))
        ps_gram = ctx.enter_context(tc.tile_pool(name="ps_gram", bufs=1, space="PSUM"))
        ps_b = ps_gram
        ps_fo = ctx.enter_context(tc.tile_pool(name="ps_fo", bufs=2, space="PSUM"))
        dram = ctx.enter_context(tc.tile_pool(name="dram", bufs=1, space="DRAM"))

        # ---- prefetch first conv weights ----
        dwt_pre = dwt_p.tile([128, 8, 2, 128], FP8, tag="dwt", name="dwt_pre")
        nc.sync.dma_start(out=dwt_pre[:].rearrange("p s k m -> p (s k m)"),
                          in_=dwt8_d[0])
        # ---- fp8 staging: dual-band padded x buffers (zero padding comes
        # from the host-prepared padded volume; only the shifted-band tails
        # are unwritten, and only read under zero weights -> memset them) ----
        xa8 = singles.tile([128, PT, PH, PW], FP8)
        xb8 = singles.tile([128, PT, PH, PW], FP8)
        xa8f = xa8[:].rearrange("p t h w -> p (t h w)")
        xb8f = xb8[:].rearrange("p t h w -> p (t h w)")
        nc.gpsimd.memset(xa8f[64:128, PFREE - 1:PFREE], 0.0)
        nc.gpsimd.memset(xb8f[64:128, PFREE - PW:PFREE], 0.0)
        nc.sync.dma_start(out=xa8f[0:64, :], in_=x8_d)
        nc.sync.dma_start(out=xa8f[64:128, 0:PFREE - 1], in_=x8_d[:, 1:])
        nc.sync.dma_start(out=xb8f[0:64, :], in_=x8_d)
        nc.sync.dma_start(out=xb8f[64:128, 0:PFREE - PW], in_=x8_d[:, PW:])

        # bf16 staging for the v-conv (loaded later; only needed at the end)
        xa = singles.tile([128, PT, PH, PW], BF16)
        xb = singles.tile([128, PT, PH, PW], BF16)
        xc = singles.tile([128, PT, PH, PW], BF16)
        xaf = xa[:].rearrange("p t h w -> p (t h w)")
        xbf = xb[:].rearrange("p t h w -> p (t h w)")
        xcf = xc[:].rearrange("p t h w -> p (t h w)")

        projt_sb = singles.tile([128, 2, DIM], F32)
        eye_sb = singles.tile([128, 128], F32)
        tsc = singles.tile([128, 2], F32)

        # dense bf16 buffers for q,k (to transpose via DMA)
        qk_dense = [dense_p.tile([128, N_LOC], BF16, tag=f"qk{m}", name=f"qk{m}")
                    for m in range(4)]

        gq_ps = [None, None]

        arbuf = singles.tile([128, 2, 130], F32)
        ssqk = singles.tile([128, 2, T], F32)

        # macro order: q0, k0, q1, k1 (qkv ch-macros 0,2,1,3); v folded through attn
        macro_order = [0, 2, 1, 3]

        def dr_rhs(s, t):
            tn, dt, h0, w0, delta = DR_SLOTS[s]
            xt = xa8 if tn == 0 else xb8
            base = xt[:, t + dt, h0:h0 + 8, w0:w0 + 64]
            return bass.AP(tensor=base.tensor, offset=base.offset,
                           ap=[list(base.ap[0]), [delta, 2],
                               list(base.ap[1]), list(base.ap[2])])

        def conv_macro(mac, mi, dwt_pre=None):
            """Folded (qkv1*dwconv) dense conv for a 128-channel macro tile:
            8 FP8 DoubleRow matmuls per t-plane (each covers 4 band/ko taps)."""
            if dwt_pre is not None:
                dwt_sb = dwt_pre
            else:
                dwt_sb = dwt_p.tile([128, 8, 2, 128], FP8, tag="dwt")
                nc.sync.dma_start(out=dwt_sb[:].rearrange("p s k m -> p (s k m)"),
                                  in_=dwt8_d[mac])

            for t in range(T):
                ps = ps_conv.tile([128, NT], F32, tag="cps")
                for s in range(8):
                    nc.tensor.matmul(ps[:], dwt_sb[:, s, :, :], dr_rhs(s, t),
                                     start=(s == 0), stop=(s == 7), perf_mode=DR)
                dst = qk_dense[mac][:, t * NT:(t + 1) * NT]
                if t % 2 == 0:
                    nc.vector.tensor_copy(out=dst, in_=ps[:])
                else:
                    nc.scalar.copy(out=dst, in_=ps[:])
                if mac >= 2:
                    # ssq_k on DVE from the evacuated bf16 plane
                    scr = small_p.tile([128, NT], F32, tag="ttr")
                    nc.vector.tensor_mul(scr[:], dst, dst)
                    nc.vector.tensor_reduce(out=ssqk[:, mac - 2, t:t + 1], in_=scr[:],
                                            axis=mybir.AxisListType.X,
                                            op=mybir.AluOpType.add)

        def gram_pair(p):
            gq_ps[p] = ps_gram.tile([128, 256], F32, tag="gq", name=f"gq{p}")
            for g in range(N_LOC // 1024):
                ct = ct_p.tile([128, 8, 256], BF16, tag="ct", name=f"ct{p}_{g}")
                sl = slice(g * 1024, (g + 1) * 1024)
                nc.sync.dma_start(out=ct[:, :, 0:128], in_=qk_dense[p][:, sl],
                                  transpose=True)
                nc.sync.dma_start(out=ct[:, :, 128:256], in_=qk_dense[2 + p][:, sl],
                                  transpose=True)
                for j in range(8):
                    jj = g * 8 + j
                    nc.tensor.matmul(gq_ps[p][:], ct[:, j, 0:128], ct[:, j, :],
                                     start=(jj == 0), stop=(jj == N_LOC // 128 - 1))

        def extract_pair(p):
            """S block + diagonals of pair p into arbuf[:, p, :]."""
            nc.vector.tensor_copy(out=arbuf[:, p, 0:128], in_=gq_ps[p][:, 128:256])
            scr = small_p.tile([128, 128], F32, tag="scr")
            nc.vector.tensor_mul(scr[:], gq_ps[p][:, 0:128], eye_sb[:])
            nc.vector.tensor_reduce(out=arbuf[:, p, 128:129], in_=scr[:],
                                    axis=mybir.AxisListType.X, op=mybir.AluOpType.add)
            nc.vector.tensor_reduce(out=arbuf[:, p, 129:130], in_=ssqk[:, p, :],
                                    axis=mybir.AxisListType.X, op=mybir.AluOpType.add)

        # ---- conv phase with gram interleaved ----
        ar_in = [dram.tile([128, 130], F32, name=f"ar_in{p}") for p in range(2)]
        ar_out = [dram.tile([128, 130], F32, name=f"ar_out{p}") for p in range(2)]
        gar = singles.tile([128, 2, 130], F32)

        def launch_ar(p):
            nc.gpsimd.dma_start(out=ar_in[p][:], in_=arbuf[:, p, :])
            if sim:
                nc.gpsimd.dma_start(out=ar_out[p][:], in_=ar_in[p][:])
            else:
                nc.gpsimd.collective_compute(
                    "AllReduce", mybir.AluOpType.add,
                    replica_groups=[list(range(N_CORES))],
                    ins=[ar_in[p].opt()], outs=[ar_out[p].opt()])
            nc.gpsimd.dma_start(out=gar[:, p, :], in_=ar_out[p][:])

        if stop_after == "inputs":
            nc.gpsimd.dma_start(out=out_d[:, 0], in_=xa8[0:64, 0, 0:8, 0:64])
            return
        for mi, mac in enumerate(macro_order):
            if mi == 3 and stop_after != "convonly":
                conv_macro(mac, mi)
                gram_pair(1)
                extract_pair(1)
                launch_ar(1)
            elif mi != 3:
                conv_macro(mac, mi, dwt_pre=dwt_pre if mi == 0 else None)
            else:
                conv_macro(mac, mi)
            if mi == 0:
                # bf16 staging for the v-conv; deferred so fp8 staging and the
                # first conv macro own the DMA queues at kernel start
                nc.sync.dma_start(out=xaf[0:64, :], in_=x_d)
                nc.sync.dma_start(out=xaf[64:128, 0:PFREE - 1], in_=x_d[:, 1:])
                nc.sync.dma_start(out=xbf[0:64, :], in_=x_d)
                nc.sync.dma_start(out=xbf[64:128, 0:PFREE - PW], in_=x_d[:, PW:])
                nc.sync.dma_start(out=xcf[0:64, :], in_=x_d)
                nc.sync.dma_start(out=xcf[64:128, 0:PFREE - PLANE], in_=x_d[:, PLANE:])
            if mi == 1:
                # deferred small input loads (needed only post-conv)
                nc.sync.dma_start(out=projt_sb[:], in_=projt_d)
                nc.sync.dma_start(out=eye_sb[:], in_=eye_d)
                for p_ in range(2):
                    for hf_ in range(2):
                        src_ = bass.AP(tensor=temp_d.tensor, offset=2 * p_ + hf_,
                                       ap=[[0, 64], [1, 1]])
                        nc.sync.dma_start(out=tsc[hf_ * 64:(hf_ + 1) * 64, p_:p_ + 1],
                                          in_=src_)
            if mi == 2 and stop_after != "convonly":
                gram_pair(0)
                extract_pair(0)
                launch_ar(0)

        if stop_after in ("conv", "convonly"):
            return
        # ---- per-pair: normalization, softmax, B ----
        rno = singles.tile([128, 2, 2], F32)
        rqs = singles.tile([128, 2], F32)
        rk_d = [dram.tile([128, 1], F32, name=f"rk_d{p}") for p in range(2)]
        rkb = singles.tile([128, 2, 128], F32)
        bt_sb = [singles.tile([128, DIM], BF16, tag=f"bt{p}", name=f"bt{p}")
                 for p in range(2)]
        for p in range(2):
            nc.scalar.activation(out=rno[:, p, :], in_=gar[:, p, 128:130],
                                 func=mybir.ActivationFunctionType.Sqrt)
            nc.vector.reciprocal(out=rno[:, p, :], in_=rno[:, p, :])
            nc.vector.tensor_mul(rqs[:, p:p + 1], rno[:, p, 0:1], tsc[:, p:p + 1])
            nc.sync.dma_start(out=rk_d[p][:], in_=rno[:, p, 1:2])
            src = bass.AP(tensor=rk_d[p].tensor, offset=rk_d[p].offset,
                          ap=[[0, 128], [1, 128]])
            nc.sync.dma_start(out=rkb[:, p, :], in_=src)

            lg = small_p.tile([128, 128], F32, tag="lg")
            nc.vector.tensor_mul(lg[:], gar[:, p, 0:128], rkb[:, p, :])
            nc.vector.tensor_scalar_mul(lg[:], lg[:], rqs[:, p:p + 1])
            btp = ps_b.tile([128, DIM], F32, tag="gk", name=f"btp{p}")
            mx = small_p.tile([128, 1], F32, tag="mx")
            at = small_p.tile([128, 64], F32, tag="at")
            sm = small_p.tile([128, 1], F32, tag="sm")
            for hf in range(2):
                hs = slice(hf * 64, (hf + 1) * 64)
                sub = lg[hs, hf * 64:(hf + 1) * 64]
                nc.vector.tensor_reduce(out=mx[hs], in_=sub, axis=mybir.AxisListType.X,
                                        op=mybir.AluOpType.max, negate=True)
                nc.scalar.activation(out=at[hs], in_=sub,
                                     func=mybir.ActivationFunctionType.Exp,
                                     bias=mx[hs], scale=1.0)
                nc.vector.tensor_reduce(out=sm[hs], in_=at[hs], axis=mybir.AxisListType.X,
                                        op=mybir.AluOpType.add)
                nc.vector.reciprocal(out=sm[hs], in_=sm[hs])
                nc.vector.tensor_scalar_mul(at[hs], at[hs], sm[hs])
                # B_h^T = attn_h^T @ projT_h  (partitions hf*64.. aligned throughout)
                nc.tensor.matmul(btp[hs, :], at[hs], projt_sb[hs, p, :],
                                 start=True, stop=True)
            nc.vector.tensor_copy(out=bt_sb[p][:], in_=btp[:])

        if stop_after == "softmax":
            return
        # ---- compose G = (B o Mfold_v): per slot GT[(band,c), e] ----
        dwtv_sb = singles.tile([128, 14, 2, 128], BF16)
        nc.sync.dma_start(out=dwtv_sb[:], in_=dwtv_d.rearrange("s p k m -> k s p m"))
        gv = singles.tile([128, 14, DIM], BF16)
        for slot in range(14):
            gts = ps_gram.tile([128, DIM], F32, tag=("gq" if slot % 2 == 0 else "gk"),
                               name=f"gts{slot}")
            nc.tensor.matmul(gts[:], dwtv_sb[:, slot, 0, :], bt_sb[0][:],
                             start=True, stop=False)
            nc.tensor.matmul(gts[:], dwtv_sb[:, slot, 1, :], bt_sb[1][:],
                             start=False, stop=True)
            nc.vector.tensor_copy(out=gv[:, slot, :], in_=gts[:])

        # ---- v-conv: out = G * x; two t-planes per pass via PE column tiling ----
        def vslot_rhs(t, slot):
            if slot < 9:
                dti, dhi = slot // 3, slot % 3
                return xa[:, t + dti, dhi:dhi + 8, 1:65]
            if slot < 12:
                return xb[:, t + (slot - 9), 0:8, 3:67]
            if slot == 12:
                return xc[:, t, 2:10, 3:67]
            return xa[:, t + 2, 2:10, 3:67]

        for tp in range(T // 2):
            t0, t1 = 2 * tp, 2 * tp + 1
            fo = ps_fo.tile([128, NT], F32, tag="fo")
            for slot in range(14):
                nc.tensor.matmul(fo[0:64, :], gv[:, slot, :], vslot_rhs(t0, slot),
                                 start=(slot == 0), stop=(slot == 13))
                nc.tensor.matmul(fo[64:128, :], gv[:, slot, :], vslot_rhs(t1, slot),
                                 start=(slot == 0), stop=(slot == 13))
            ot = out_p.tile([128, NT], F32, tag="ot")
            if tp % 2 == 0:
                nc.vector.tensor_copy(out=ot[:], in_=fo[:])
            else:
                nc.scalar.copy(out=ot[:], in_=fo[:])
            nc.sync.dma_start(out=out_d[:, t0],
                              in_=ot[0:64].rearrange("p (h w) -> p h w", h=HL))
            nc.sync.dma_start(out=out_d[:, t1],
                              in_=ot[64:128].rearrange("p (h w) -> p h w", h=HL))


def _prep_inputs(x, qkv_w, dw_w, proj_w, temperature):
    """Host-side sharding + weight layout."""
    b, c, t, h, w = x.shape
    w1 = qkv_w.reshape(C3H, DIM).astype(np.float64)   # (768, 64)
    dw = dw_w.reshape(C3H, 4, 3, 3, 3).astype(np.float64)
    # folded conv: M[o, c, dti, dhi, dwi] = sum_j dw[o, j, taps] * w1[4*(o//4)+j, c]
    j_idx = (np.arange(C3H) // 4) * 4
    w1g = w1[j_idx[:, None] + np.arange(4)[None, :], :]      # (768, 4, 64)
    mfold = np.einsum("ojtuv,ojc->octuv", dw, w1g)           # (768, 64, 3,3,3)

    # fp8 q,k weights: per-output-channel scale (cancels in L2 normalization)
    qk = mfold[:512]                                          # (512, 64, 3,3,3)
    scale = 224.0 / np.maximum(np.abs(qk).max(axis=(1, 2, 3, 4)), 1e-30)
    qk_s = qk * scale[:, None, None, None, None]
    dwt8 = np.zeros((4, 128, 8, 2, 128), np.float32)
    for mac in range(4):
        osl = slice(mac * 128, (mac + 1) * 128)
        for s in range(8):
            for ko in range(2):
                for bnd in range(2):
                    tap = DR_TAPS[s][ko][bnd]
                    if tap is None:
                        continue
                    dt_, dh_, dw_ = tap
                    dwt8[mac, bnd * 64:(bnd + 1) * 64, s, ko, :] = \
                        qk_s[osl, :, dt_, dh_, dw_].T
    dwt8 = dwt8.reshape(4, 128, 2048).astype(ml_dtypes.float8_e4m3)

    # v-conv fold weights (bf16, 14 dual-band slots like the baseline layout)
    slots = []
    for dti in range(3):
        for dhi in range(3):
            slots.append(((dti, dhi, 0), (dti, dhi, 1)))     # A-pairs
    for dti in range(3):
        slots.append(((dti, 0, 2), (dti, 1, 2)))             # B-pairs (h-shift band)
    slots.append(((0, 2, 2), (1, 2, 2)))                     # C-pair (t-shift band)
    slots.append(((2, 2, 2), None))                          # single
    # dwtv[s, p, o, 64b + c] = mfold[512 + 128p + o, c, tap(s, b)]
    dwtv = np.zeros((14, 2, 128, 128), dtype=np.float32)
    for si, (tap0, tap1) in enumerate(slots):
        for p in range(2):
            osl = slice(512 + p * 128, 512 + (p + 1) * 128)
            dwtv[si, p, :, 0:64] = mfold[osl, :, tap0[0], tap0[1], tap0[2]]
            if tap1 is not None:
                dwtv[si, p, :, 64:128] = mfold[osl, :, tap1[0], tap1[1], tap1[2]]
    dwtv = dwtv.astype(ml_dtypes.bfloat16)

    pw = proj_w.reshape(DIM, HEADS, DIM)              # (e, h, c)
    # projt[hf*64+c, p, e] = proj_w[e, (2p+hf)*64 + c]
    projt = np.zeros((128, 2, DIM), dtype=np.float32)
    for p in range(2):
        for hf in range(2):
            projt[hf * 64:(hf + 1) * 64, p, :] = pw[:, 2 * p + hf, :].T
    temp = np.asarray(temperature, dtype=np.float32).reshape(HEADS)
    eye = np.eye(128, dtype=np.float32)

    xp = np.zeros((c, t, h + 2, w), dtype=np.float32)
    xp[:, :, 1:h + 1, :] = x[0]
    in_maps = []
    for i in range(N_CORES):
        xs = np.zeros((c, PT, PH, PW), dtype=np.float32)
        xs[:, 1:T + 1, :, 2:W + 2] = xp[:, :, i * HL:i * HL + HLH, :]
        xs = xs.reshape(c, PFREE)
        in_maps.append({"x": xs.astype(ml_dtypes.bfloat16),
                        "x8": (xs * 16.0).astype(ml_dtypes.float8_e4m3),
                        "dwt8": dwt8, "dwtv": dwtv, "projt": projt,
                        "temp": temp, "eye": eye})
    return in_maps


def kernel(x, qkv_w, dw_w, proj_w, temperature, _trace=False):
    if "nc" not in _CACHE:
        _CACHE["nc"] = _build()
    nc = _CACHE["nc"]
    in_maps = _prep_inputs(np.asarray(x, np.float32), np.asarray(qkv_w, np.float32),
                           np.asarray(dw_w, np.float32), np.asarray(proj_w, np.float32),
                           np.asarray(temperature, np.float32))
    kw = {}
    if _trace:
        kw = dict(trace=True, stitch_traces=True, trace_cores=list(range(N_CORES)))
    res = run_bass_kernel_spmd(nc, in_maps, core_ids=list(range(N_CORES)), **kw)
    _CACHE["last_res"] = res
    out = np.zeros((1, DIM, T, H, W), dtype=np.float32)
    for i in range(N_CORES):
        out[0, :, :, i * HL:(i + 1) * HL, :] = res.results[i]["out"]
    return out
